# revision 49
# baseline (speedup 1.0000x reference)
"""Trainium2 Bass kernel for nn_CapsuleLayer (dynamic routing, 3 iterations).

Full problem:
  x:  [64, 2048, 16] f32;  route_weights: [32, 2048, 16, 32] f32
  priors[b,c,n,o] = sum_i x[b,n,i] * rw[c,n,i,o]
  3 rounds of routing-by-agreement (softmax over n=2048); output [64, 32, 32].

Sharding: capsule-parallel over 8 cores (CL=4 caps/core), batch replicated.

Per-core dataflow (engine-balanced; B-logits kept n-partitioned):
  phase A   s1T[(cl,o), b]  = sum_{n,i} rw4^T x         lhsT=rw4 (M=128), stream xT b-cols
  e'-pass   e'[(2i,b), n]   = sum_o outwBD * rwt        as baseline (stream rwt n-cols)
            conv            = e' f32->f16               ACT copies PSUM->SBUF
            xe              = conv * x2e                DVE f16 2x mul
            dBT[n128, b]    = selT matmul               lhsT=xe chunk (M=n!), rhs=s2sel
            BstT[n,cl,g,b] += dBT                       DVE adds (f32)
  softmax   w~T = exp(BstT - shift) bf16 (constant shift; no col-max needed:
            iter2 B in [-7,8], iter3 B in [-34,47] -> shift 0 / 25)
            Z[b,cl] via ones-matmul over w~T
  s~-pass   xw = xT * w~T (bf16, DVE/Pool split)
            s~[b,o] per cl: lhsT=xw (stationary), stream rw4 o-cols (32 rows)
  squash    alpha = sqrt(sq)/(den+sq); den = N^2 (it1) or Z^2 (it2,3)
"""

import sys

sys.path.insert(0, "/opt/trn_rl_repo")

import numpy as np
import ml_dtypes

import concourse.bass as bass
import concourse.bacc as bacc
import concourse.tile as tile
import concourse.mybir as mybir
from concourse.bass_utils import run_bass_kernel_spmd

F16 = mybir.dt.float16
BF16 = mybir.dt.bfloat16
F32 = mybir.dt.float32
AX = mybir.AxisListType
ALU = mybir.AluOpType
ACTF = mybir.ActivationFunctionType

B, C, N, ID, OD = 64, 32, 2048, 16, 32
NCORES = 8
CL = C // NCORES          # 4 local caps per core
G = N // 128              # 16 n-blocks of 128
NU = N // 512             # 4 n-chunks of 512

SHIFT = {2: 0.0, 3: 25.0}   # constant softmax shift per routing iteration

f16 = np.float16
bf16 = ml_dtypes.bfloat16


def _build(n_rounds=3, debug=False):
    nc = bacc.Bacc("TRN2")

    xT_d = nc.dram_tensor("xT", [128, G, ID, B], F16, kind="ExternalInput")
    rw4_d = nc.dram_tensor("rw4", [128, G, ID, 128], F16, kind="ExternalInput")
    rwt_d = nc.dram_tensor("rwt", [CL, 128, 4, N], F16, kind="ExternalInput")
    x2e_d = nc.dram_tensor("x2e", [128, 8, N], F16, kind="ExternalInput")
    out_d = nc.dram_tensor("out", [B, CL * OD], F32, kind="ExternalOutput")
    if debug:
        dbg_B_d = nc.dram_tensor("dbg_B", [128, CL, G, B], F32, kind="ExternalOutput")
        dbg_w_d = nc.dram_tensor("dbg_w", [128, CL, G, B], BF16, kind="ExternalOutput")
        dbg_Z_d = nc.dram_tensor("dbg_Z", [B, CL], F32, kind="ExternalOutput")

    # selector: [128, B]; col b has 1 at rows b and 64+b (sums i-pairs)
    s2sel_d = nc.inline_tensor(np.tile(np.eye(B, dtype=f16), (2, 1)), name="s2sel")
    ones1_d = nc.inline_tensor(np.ones((128, 1), dtype=bf16), name="ones1")
    # block eye [128, 4] f32: col cl has 1 at partitions 32cl..32cl+31
    be4_d = nc.inline_tensor(
        np.repeat(np.eye(4, dtype=np.float32), 32, axis=0), name="be4"
    )
    # e4 [4, 128] f32: row cl has 1s at cols 32cl..32cl+31
    e4_d = nc.inline_tensor(
        np.repeat(np.eye(4, dtype=np.float32), 32, axis=1), name="e4"
    )
    id64_d = nc.inline_tensor(np.eye(64, dtype=np.float32), name="id64")
    id128_d = nc.inline_tensor(np.eye(128, dtype=np.float32), name="id128")

    with tile.TileContext(nc) as tc:
        with (
            tc.tile_pool(name="res", bufs=1) as res,
            tc.tile_pool(name="stream", bufs=2) as stream,
            tc.tile_pool(name="small", bufs=2) as small,
            tc.tile_pool(name="ppe", bufs=3, space="PSUM") as ppe,
            tc.tile_pool(name="pdb", bufs=1, space="PSUM") as pdbp,
            tc.tile_pool(name="psml", bufs=1, space="PSUM") as psml,
        ):
            xT = res.tile([128, G, ID, B], F16)
            rw4 = res.tile([128, G, ID, 128], F16)
            x2e = res.tile([128, 8, N], F16)
            s2sel = res.tile([128, B], F16)
            ones1 = res.tile([128, 1], BF16)
            be4 = res.tile([128, 4], F32)
            e4 = res.tile([4, 128], F32)
            id64 = res.tile([64, 64], F32)
            id128 = res.tile([128, 128], F32)
            BstT = res.tile([128, CL, G, B], F32)
            wT = res.tile([128, CL, G, B], BF16)
            outwBD = res.tile([128, CL, 128], F16)
            outf = res.tile([B, CL, OD], F32)

            # One shared PSUM bank "pq" hosts phase-A accumulators and,
            # during iterations, the s~ accumulators, Z and the out
            # transposes.  Matmul start=True lazily zeroes the whole 2KB
            # zero-region (pending-zero); non-matmul reads are unaffected,
            # and a start=False matmul whose bytes are pending simply
            # overwrites.  So: one start=True per capsule-pair (the first
            # s~-mm), everything else start=False + skip_group_check.
            pq = psml.tile([128, 512], F32, tag="pq")

            def pstp(cl):
                return pq[0:B, 128 + 32 * cl:160 + 32 * cl]

            nc.sync.dma_start(out=s2sel, in_=s2sel_d[:, :])
            nc.sync.dma_start(out=ones1, in_=ones1_d[:, :])
            nc.sync.dma_start(out=be4, in_=be4_d[:, :])
            nc.sync.dma_start(out=e4, in_=e4_d[:, :])
            nc.sync.dma_start(out=id64, in_=id64_d[:, :])
            nc.sync.dma_start(out=id128, in_=id128_d[:, :])
            # inputs for phase A, interleaved chunks so compute can start early
            for gq in range(4):
                nc.sync.dma_start(
                    out=xT[:, 4 * gq:4 * gq + 4], in_=xT_d[:, 4 * gq:4 * gq + 4]
                )
                nc.sync.dma_start(
                    out=rw4[:, 4 * gq:4 * gq + 2], in_=rw4_d[:, 4 * gq:4 * gq + 2]
                )
                nc.sync.dma_start(
                    out=rw4[:, 4 * gq + 2:4 * gq + 4],
                    in_=rw4_d[:, 4 * gq + 2:4 * gq + 4]
                )
            nc.gpsimd.memset(BstT, 0.0)
            nc.gpsimd.memset(outwBD, 0.0)
            nshift3 = res.tile([128, 1], F32)
            nc.gpsimd.memset(nshift3, -SHIFT[3])

            # ---------------- phase A: s1T[(cl,o), b] ----------------
            ps1 = pq[:, 0:B]
            for g in range(G):
                for i in range(ID):
                    nc.tensor.matmul(
                        ps1,
                        rw4[:, g, i],
                        xT[:, g, i],
                        start=(g == 0 and i == 0),
                        stop=(g == G - 1 and i == ID - 1),
                    )

            def build_outw(outT_ap, cl):
                # outT [32, B] fp16 (o-part, b) for capsule cl -> outwBD blocks:
                # rows (64h + 32isub + ... wait layout: [(2isub x 32o?) ...]
                # outwBD[64h+32s : 64h+32s+32, cl, 64s:64s+64] = outT
                for h in range(2):
                    for s in range(2):
                        nc.gpsimd.tensor_copy(
                            outwBD[64 * h + 32 * s:64 * h + 32 * s + 32,
                                   cl, 64 * s:64 * s + 64],
                            outT_ap,
                        )

            # squash of s1T (uniform routing; divide-by-N folded into alpha)
            sq1 = small.tile([128, B], F32, tag="sq1")
            nc.scalar.activation(sq1, ps1, ACTF.Square)
            psq1 = pq[0:4, B:2 * B]
            nc.tensor.matmul(psq1, be4, sq1, start=True, stop=True)
            sq4 = small.tile([4, B], F32, tag="sq4")
            rs4 = small.tile([4, B], F32, tag="rs4")
            den4 = small.tile([4, B], F32, tag="den4")
            al4 = small.tile([4, B], F32, tag="al4")
            nc.scalar.copy(sq4, psq1)
            nc.scalar.activation(rs4, sq4, ACTF.Ln)
            nc.scalar.activation(rs4, rs4, ACTF.Exp, bias=0.0, scale=0.5)
            nc.vector.tensor_scalar_add(den4, sq4, float(N) * float(N))
            nc.vector.reciprocal(den4, den4)
            nc.vector.tensor_mul(al4, rs4, den4)
            pal = pq[:, 2 * B:3 * B]
            nc.tensor.matmul(pal, e4, al4, start=True, stop=True)
            pal_sb = small.tile([128, B], F32, tag="pal_sb")
            nc.scalar.copy(pal_sb, pal)
            outT1 = small.tile([128, B], F16, tag="outT")
            nc.vector.tensor_mul(outT1, ps1, pal_sb)
            for cl in range(CL):
                build_outw(outT1[32 * cl:32 * cl + 32], cl)

            if n_rounds == 1:
                # out[b, (cl,o)] = transpose of outT1... recompute in f32
                o1f = small.tile([128, B], F32, tag="o1f")
                nc.vector.tensor_mul(o1f, ps1, pal_sb)
                pot = pq[0:B, B:B + 128]
                nc.tensor.transpose(pot, o1f, id128)
                nc.scalar.copy(outf.rearrange("b c o -> b (c o)"), pot)
                nc.sync.dma_start(
                    out=out_d[:, :], in_=outf.rearrange("b c o -> b (c o)")
                )

            # ---------------- routing iterations 2..n ----------------
            # u-granular fused pipeline, two capsule chains interleaved:
            # per (cl, u): e' -> conv -> xe -> selT -> B-add -> exp -> xw ->
            # s~-mms.  The cl-pair interleave keeps every engine fed with an
            # independent chain.  Z + squash batched per pair (sqrt via
            # exp(0.5 ln x) keeps ACT on one table set).
            def eprime_unit(it, cl, u, final, shift, dbt):
                rt = stream.tile([128, 4, 512], F16, tag="rt", bufs=3)
                nc.sync.dma_start(
                    out=rt, in_=rwt_d[cl, :, :, 512 * u:512 * u + 512]
                )
                if it == 2 and cl == 0:
                    nc.sync.dma_start(
                        out=x2e[:, :, 512 * u:512 * u + 512],
                        in_=x2e_d[:, :, 512 * u:512 * u + 512],
                    )
                xes = []
                for ib in range(4):
                    pe = ppe.tile([128, 2, 512], F32, tag="pe")
                    for h in range(2):
                        nc.tensor.matmul(
                            pe[:, h],
                            outwBD[64 * h:64 * h + 64, cl],
                            rt[64 * h:64 * h + 64, ib],
                            start=True,
                            stop=True,
                        )
                    xe = stream.tile([128, 2, 512], F16, tag="xe", bufs=7)
                    x2s = x2e[:, 2 * ib:2 * ib + 2, 512 * u:512 * u + 512]
                    cv = stream.tile([128, 2, 512], F16, tag="cv", bufs=3)
                    nc.scalar.copy(cv, pe)
                    xeng = nc.gpsimd if (ib == 2 and u == 1) else nc.vector
                    xeng.tensor_mul(xe, cv, x2s)
                    xes.append(xe)
                for cch in range(4):
                    for ib in range(4):
                        for j in range(2):
                            nc.tensor.matmul(
                                dbt[:, cch],
                                xes[ib][:, j, 128 * cch:128 * cch + 128],
                                s2sel,
                                start=(ib == 0 and j == 0),
                                stop=(ib == 3 and j == 1),
                            )
                nc.vector.tensor_add(
                    BstT[:, cl, 4 * u:4 * u + 4],
                    BstT[:, cl, 4 * u:4 * u + 4], dbt
                )
                # softmax numerator for this u-chunk (elementwise)
                nc.scalar.activation(
                    wT[:, cl, 4 * u:4 * u + 4],
                    BstT[:, cl, 4 * u:4 * u + 4], ACTF.Exp,
                    bias=(0.0 if shift == 0.0 else nshift3), scale=1.0,
                )
                # s~ partial sums for this u-chunk
                for gp in (2 * u, 2 * u + 1):
                    xw = stream.tile([128, 2, ID, B], BF16, tag="xw", bufs=3)
                    weng = nc.vector if (final and cl == CL - 1
                                         and gp % 2 == 1) else nc.gpsimd
                    weng.tensor_mul(
                        xw,
                        xT[:, 2 * gp:2 * gp + 2],
                        wT[:, cl, 2 * gp:2 * gp + 2]
                        .unsqueeze(2).broadcast_to((128, 2, ID, B)),
                    )
                    for gg in range(2):
                        g = 2 * gp + gg
                        for i in range(ID):
                            nc.tensor.matmul(
                                pstp(cl),
                                xw[:, gg, i],
                                rw4[:, g, i, 32 * cl:32 * cl + 32],
                                start=(gp == 0 and gg == 0 and i == 0
                                       and u == 0),
                                stop=(gp == 2 * u + 1 and gg == 1
                                      and i == ID - 1 and u == NU - 1),
                            )

            for it in range(2, n_rounds + 1):
                final = it == n_rounds
                shift = SHIFT[it]
                zps = pq[0:B, 256 + 4 * (it % 2):260 + 4 * (it % 2)]
                sqa = small.tile([B, CL], F32, tag="sqa", bufs=2)
                for pair in range(2):
                    cls = (2 * pair, 2 * pair + 1)
                    snn = small.tile([B, 2, OD], F32, tag="snn", bufs=2)
                    for cl in cls:
                        for u in range(NU):
                            dbt = pdbp.tile([128, 4, B], F32, tag="dbt")
                            eprime_unit(it, cl, u, final, shift, dbt)
                        # Z over the full row for this capsule
                        for g in range(G):
                            nc.tensor.matmul(
                                zps[:, cl:cl + 1],
                                wT[:, cl, g],
                                ones1,
                                start=(g == 0),
                                stop=(g == G - 1),
                            )
                        # normalize by Z per partition BEFORE squash so the
                        # ACT ln/exp tables see moderate ranges (sq <= ~1e3,
                        # den = 1+sq) -- the compiled tables lose precision
                        # on the unnormalized 1e28-scale values
                        zsb = small.tile([B, 1], F32, tag="zsb", bufs=4)
                        nc.vector.reciprocal(zsb, zps[:, cl:cl + 1])
                        nc.vector.tensor_mul(
                            snn[:, cl % 2], pstp(cl), zsb.broadcast_to((B, OD))
                        )
                        sqs = small.tile([B, OD], F32, tag="sqs", bufs=4)
                        nc.scalar.activation(sqs, snn[:, cl % 2], ACTF.Square,
                                             accum_out=sqa[:, cl:cl + 1])
                    # squash for the pair; alpha = sqrt(sq)/(1+sq)
                    lo, hi = cls
                    lnq = small.tile([B, 2], F32, tag="lnq", bufs=4)
                    nc.scalar.activation(lnq, sqa[:, lo:hi + 1], ACTF.Ln)
                    rs = small.tile([B, 2], F32, tag="rs", bufs=4)
                    nc.scalar.activation(rs, lnq, ACTF.Exp, bias=0.0,
                                         scale=0.5)
                    den = small.tile([B, 2], F32, tag="den", bufs=4)
                    nc.vector.tensor_scalar_add(den, sqa[:, lo:hi + 1], 1.0)
                    nc.vector.reciprocal(den, den)
                    al2 = small.tile([B, 2], F32, tag="al2", bufs=4)
                    nc.vector.tensor_mul(al2, rs, den)
                    if final:
                        for c2 in cls:
                            nc.vector.tensor_mul(
                                outf[:, c2], snn[:, c2 % 2],
                                al2[:, c2 - lo:c2 - lo + 1]
                                .broadcast_to((B, OD)),
                            )
                        nc.sync.dma_start(
                            out=out_d[:, 64 * pair:64 * pair + 64],
                            in_=outf[:, lo:hi + 1]
                            .rearrange("b c o -> b (c o)"),
                        )
                    else:
                        for c2 in cls:
                            ob = small.tile([B, OD], F32, tag="ob", bufs=4)
                            nc.vector.tensor_mul(
                                ob, snn[:, c2 % 2],
                                al2[:, c2 - lo:c2 - lo + 1]
                                .broadcast_to((B, OD))
                            )
                            pto = pq[0:OD, 264 + 64 * (c2 % 2):
                                     264 + 64 * (c2 % 2) + B]
                            nc.tensor.transpose(pto, ob, id64)
                            otn = small.tile([OD, B], F16, tag="otn",
                                             bufs=4)
                            nc.scalar.copy(otn, pto)
                            build_outw(otn, c2)
                if debug and final:
                    nc.sync.dma_start(out=dbg_B_d[:, :, :, :], in_=BstT)
                    nc.sync.dma_start(out=dbg_w_d[:, :, :, :], in_=wT)
                    zc = small.tile([B, CL], F32, tag="zc")
                    nc.scalar.copy(zc, zps)
                    nc.sync.dma_start(out=dbg_Z_d[:, :], in_=zc)

    return nc


_NC_CACHE = {}


def _get_nc(n_rounds=3, debug=False):
    key = (n_rounds, debug)
    if key not in _NC_CACHE:
        nc = _build(n_rounds=n_rounds, debug=debug)
        nc.finalize()
        _NC_CACHE[key] = nc
    return _NC_CACHE[key]


def make_in_maps(x, rw):
    x = np.asarray(x, dtype=np.float32)
    rw = np.asarray(rw, dtype=np.float32)
    # xT [128, G, ID, B]: (p, g, i, b) = x[b, 128g+p, i]
    xT_h = np.ascontiguousarray(
        x.reshape(B, G, 128, ID).transpose(2, 1, 3, 0).astype(f16)
    )
    # x2e [128, 8, N]: q<64 -> x[q, n, 2k]; q>=64 -> x[q-64, n, 2k+1]
    x2e_h = np.empty((128, 8, N), dtype=f16)
    xt = x.transpose(2, 0, 1).astype(f16)  # [i, b, n]
    for k in range(8):
        x2e_h[:64, k] = xt[2 * k]
        x2e_h[64:, k] = xt[2 * k + 1]

    in_maps = []
    for core in range(NCORES):
        rws = rw[CL * core: CL * core + CL]  # [4, N, ID, OD]
        rw4_h = np.ascontiguousarray(
            rws.reshape(CL, G, 128, ID, OD).transpose(2, 1, 3, 0, 4)
            .reshape(128, G, ID, CL * OD).astype(f16)
        )
        # rwt [cl, 32r+o, ib, n] = rw[cl, n, 4ib+r, o]
        rwt_h = np.ascontiguousarray(
            rws.reshape(CL, N, 4, 4, OD).transpose(0, 3, 4, 2, 1)
            .reshape(CL, 128, 4, N).astype(f16)
        )
        in_maps.append({"xT": xT_h, "rw4": rw4_h, "rwt": rwt_h, "x2e": x2e_h})
    return in_maps


def kernel(x, route_weights, ncores=NCORES, trace=False, n_rounds=3,
           debug=False):
    in_maps = make_in_maps(x, route_weights)
    nc = _get_nc(n_rounds=n_rounds, debug=debug)
    res = run_bass_kernel_spmd(nc, in_maps[:ncores], core_ids=list(range(ncores)),
                               trace=trace)
    if trace:
        print(f"HW exec time: {res.exec_time_ns} ns")
    if debug:
        return res.results
    outs = [r["out"].reshape(B, CL, OD) for r in res.results]
    return np.concatenate(outs, axis=1).astype(np.float32)


if __name__ == "__main__":
    rng = np.random.default_rng(0)
    x = rng.standard_normal((B, N, ID), dtype=np.float32)
    rw = rng.standard_normal((C, N, ID, OD), dtype=np.float32)
    out = kernel(x, rw)
    print(out.shape, out.dtype, float(np.abs(out).mean()))


# revision 50
# speedup vs baseline: 1.0639x; 1.0639x over previous
"""Trainium2 Bass kernel for nn_CapsuleLayer (dynamic routing, 3 iterations).

Full problem:
  x:  [64, 2048, 16] f32;  route_weights: [32, 2048, 16, 32] f32
  priors[b,c,n,o] = sum_i x[b,n,i] * rw[c,n,i,o]
  3 rounds of routing-by-agreement (softmax over n=2048); output [64, 32, 32].

Sharding: capsule-parallel over 8 cores (CL=4 caps/core), batch replicated.

Per-core dataflow (engine-balanced; B-logits kept n-partitioned):
  phase A   s1T[(cl,o), b]  = sum_{n,i} rw4^T x         lhsT=rw4 (M=128), stream xT b-cols
  e'-pass   e'[(2i,b), n]   = sum_o outwBD * rwt        as baseline (stream rwt n-cols)
            conv            = e' f32->f16               ACT copies PSUM->SBUF
            xe              = conv * x2e                DVE f16 2x mul
            dBT[n128, b]    = selT matmul               lhsT=xe chunk (M=n!), rhs=s2sel
            BstT[n,cl,g,b] += dBT                       DVE adds (f32)
  softmax   w~T = exp(BstT - shift) bf16 (constant shift; no col-max needed:
            iter2 B in [-7,8], iter3 B in [-34,47] -> shift 0 / 25)
            Z[b,cl] via ones-matmul over w~T
  s~-pass   xw = xT * w~T (bf16, DVE/Pool split)
            s~[b,o] per cl: lhsT=xw (stationary), stream rw4 o-cols (32 rows)
  squash    alpha = sqrt(sq)/(den+sq); den = N^2 (it1) or Z^2 (it2,3)
"""

import sys

sys.path.insert(0, "/opt/trn_rl_repo")

import numpy as np
import ml_dtypes

import concourse.bass as bass
import concourse.bacc as bacc
import concourse.tile as tile
import concourse.mybir as mybir
from concourse.bass_utils import run_bass_kernel_spmd

F16 = mybir.dt.float16
BF16 = mybir.dt.bfloat16
F32 = mybir.dt.float32
AX = mybir.AxisListType
ALU = mybir.AluOpType
ACTF = mybir.ActivationFunctionType

B, C, N, ID, OD = 64, 32, 2048, 16, 32
NCORES = 8
CL = C // NCORES          # 4 local caps per core
G = N // 128              # 16 n-blocks of 128
NU = N // 512             # 4 n-chunks of 512

SHIFT = {2: 0.0, 3: 25.0}   # constant softmax shift per routing iteration

f16 = np.float16
bf16 = ml_dtypes.bfloat16


def _build(n_rounds=3, debug=False):
    nc = bacc.Bacc("TRN2")

    xT_d = nc.dram_tensor("xT", [128, G, ID, B], F16, kind="ExternalInput")
    rw4_d = nc.dram_tensor("rw4", [128, G, ID, 128], F16, kind="ExternalInput")
    rwt_d = nc.dram_tensor("rwt", [CL, 128, 4, N], F16, kind="ExternalInput")
    x2e_d = nc.dram_tensor("x2e", [128, 8, N], F16, kind="ExternalInput")
    out_d = nc.dram_tensor("out", [B, CL * OD], F32, kind="ExternalOutput")
    if debug:
        dbg_B_d = nc.dram_tensor("dbg_B", [128, CL, G, B], F32, kind="ExternalOutput")
        dbg_w_d = nc.dram_tensor("dbg_w", [128, CL, G, B], BF16, kind="ExternalOutput")
        dbg_Z_d = nc.dram_tensor("dbg_Z", [B, CL], F32, kind="ExternalOutput")

    # selector: [128, B]; col b has 1 at rows b and 64+b (sums i-pairs)
    s2sel_d = nc.inline_tensor(np.tile(np.eye(B, dtype=f16), (2, 1)), name="s2sel")
    ones1_d = nc.inline_tensor(np.ones((128, 1), dtype=bf16), name="ones1")
    # block eye [128, 4] f32: col cl has 1 at partitions 32cl..32cl+31
    be4_d = nc.inline_tensor(
        np.repeat(np.eye(4, dtype=np.float32), 32, axis=0), name="be4"
    )
    # e4 [4, 128] f32: row cl has 1s at cols 32cl..32cl+31
    e4_d = nc.inline_tensor(
        np.repeat(np.eye(4, dtype=np.float32), 32, axis=1), name="e4"
    )
    id64_d = nc.inline_tensor(np.eye(64, dtype=np.float32), name="id64")
    id128_d = nc.inline_tensor(np.eye(128, dtype=np.float32), name="id128")

    with tile.TileContext(nc) as tc:
        with (
            tc.tile_pool(name="res", bufs=1) as res,
            tc.tile_pool(name="stream", bufs=2) as stream,
            tc.tile_pool(name="small", bufs=2) as small,
            tc.tile_pool(name="ppe", bufs=3, space="PSUM") as ppe,
            tc.tile_pool(name="pdb", bufs=1, space="PSUM") as pdbp,
            tc.tile_pool(name="psml", bufs=1, space="PSUM") as psml,
        ):
            xT = res.tile([128, G, ID, B], F16)
            rw4 = res.tile([128, G, ID, 128], F16)
            x2e = res.tile([128, 8, N], F16)
            s2sel = res.tile([128, B], F16)
            ones1 = res.tile([128, 1], BF16)
            be4 = res.tile([128, 4], F32)
            e4 = res.tile([4, 128], F32)
            id64 = res.tile([64, 64], F32)
            id128 = res.tile([128, 128], F32)
            BstT = res.tile([128, CL, G, B], F32)
            wT = res.tile([128, CL, G, B], BF16)
            outwBD = res.tile([128, CL, 128], F16)
            outf = res.tile([B, CL, OD], F32)

            # One shared PSUM bank "pq" hosts phase-A accumulators and,
            # during iterations, the s~ accumulators, Z and the out
            # transposes.  Matmul start=True lazily zeroes the whole 2KB
            # zero-region (pending-zero); non-matmul reads are unaffected,
            # and a start=False matmul whose bytes are pending simply
            # overwrites.  So: one start=True per capsule-pair (the first
            # s~-mm), everything else start=False + skip_group_check.
            pq = psml.tile([128, 512], F32, tag="pq")

            def pstp(cl):
                return pq[0:B, 128 + 32 * cl:160 + 32 * cl]

            nc.sync.dma_start(out=s2sel, in_=s2sel_d[:, :])
            nc.sync.dma_start(out=ones1, in_=ones1_d[:, :])
            nc.sync.dma_start(out=be4, in_=be4_d[:, :])
            nc.sync.dma_start(out=e4, in_=e4_d[:, :])
            nc.sync.dma_start(out=id64, in_=id64_d[:, :])
            nc.sync.dma_start(out=id128, in_=id128_d[:, :])
            # inputs for phase A, interleaved chunks so compute can start early
            for gq in range(4):
                nc.sync.dma_start(
                    out=xT[:, 4 * gq:4 * gq + 4], in_=xT_d[:, 4 * gq:4 * gq + 4]
                )
                nc.sync.dma_start(
                    out=rw4[:, 4 * gq:4 * gq + 2], in_=rw4_d[:, 4 * gq:4 * gq + 2]
                )
                nc.sync.dma_start(
                    out=rw4[:, 4 * gq + 2:4 * gq + 4],
                    in_=rw4_d[:, 4 * gq + 2:4 * gq + 4]
                )
            nc.gpsimd.memset(BstT, 0.0)
            nc.gpsimd.memset(outwBD, 0.0)
            nshift3 = res.tile([128, 1], F32)
            nc.gpsimd.memset(nshift3, -SHIFT[3])

            # ---------------- phase A: s1T[(cl,o), b] ----------------
            ps1 = pq[:, 0:B]
            for g in range(G):
                for i in range(ID):
                    nc.tensor.matmul(
                        ps1,
                        rw4[:, g, i],
                        xT[:, g, i],
                        start=(g == 0 and i == 0),
                        stop=(g == G - 1 and i == ID - 1),
                    )

            def build_outw(outT_ap, cl):
                # outT [32, B] fp16 (o-part, b) for capsule cl -> outwBD blocks:
                # rows (64h + 32isub + ... wait layout: [(2isub x 32o?) ...]
                # outwBD[64h+32s : 64h+32s+32, cl, 64s:64s+64] = outT
                for h in range(2):
                    for s in range(2):
                        nc.gpsimd.tensor_copy(
                            outwBD[64 * h + 32 * s:64 * h + 32 * s + 32,
                                   cl, 64 * s:64 * s + 64],
                            outT_ap,
                        )

            # squash of s1T (uniform routing; divide-by-N folded into alpha)
            sq1 = small.tile([128, B], F32, tag="sq1")
            nc.scalar.activation(sq1, ps1, ACTF.Square)
            psq1 = pq[0:4, B:2 * B]
            nc.tensor.matmul(psq1, be4, sq1, start=True, stop=True)
            sq4 = small.tile([4, B], F32, tag="sq4")
            rs4 = small.tile([4, B], F32, tag="rs4")
            den4 = small.tile([4, B], F32, tag="den4")
            al4 = small.tile([4, B], F32, tag="al4")
            nc.scalar.copy(sq4, psq1)
            nc.scalar.activation(rs4, sq4, ACTF.Ln)
            nc.scalar.activation(rs4, rs4, ACTF.Exp, bias=0.0, scale=0.5)
            nc.vector.tensor_scalar_add(den4, sq4, float(N) * float(N))
            nc.vector.reciprocal(den4, den4)
            nc.vector.tensor_mul(al4, rs4, den4)
            pal = pq[:, 2 * B:3 * B]
            nc.tensor.matmul(pal, e4, al4, start=True, stop=True)
            pal_sb = small.tile([128, B], F32, tag="pal_sb")
            nc.scalar.copy(pal_sb, pal)
            outT1 = small.tile([128, B], F16, tag="outT")
            nc.vector.tensor_mul(outT1, ps1, pal_sb)
            for cl in range(CL):
                build_outw(outT1[32 * cl:32 * cl + 32], cl)

            if n_rounds == 1:
                # out[b, (cl,o)] = transpose of outT1... recompute in f32
                o1f = small.tile([128, B], F32, tag="o1f")
                nc.vector.tensor_mul(o1f, ps1, pal_sb)
                pot = pq[0:B, B:B + 128]
                nc.tensor.transpose(pot, o1f, id128)
                nc.scalar.copy(outf.rearrange("b c o -> b (c o)"), pot)
                nc.sync.dma_start(
                    out=out_d[:, :], in_=outf.rearrange("b c o -> b (c o)")
                )

            # ---------------- routing iterations 2..n ----------------
            # u-granular fused pipeline, two capsule chains interleaved:
            # per (cl, u): e' -> conv -> xe -> selT -> B-add -> exp -> xw ->
            # s~-mms.  The cl-pair interleave keeps every engine fed with an
            # independent chain.  Z + squash batched per pair (sqrt via
            # exp(0.5 ln x) keeps ACT on one table set).
            def eprime_unit(it, cl, u, final, shift, dbt):
                rt = stream.tile([128, 4, 512], F16, tag="rt", bufs=3)
                nc.sync.dma_start(
                    out=rt, in_=rwt_d[cl, :, :, 512 * u:512 * u + 512]
                )
                if it == 2 and cl == 0:
                    nc.sync.dma_start(
                        out=x2e[:, :, 512 * u:512 * u + 512],
                        in_=x2e_d[:, :, 512 * u:512 * u + 512],
                    )
                xes = []
                for ib in range(4):
                    pe = ppe.tile([128, 2, 512], F32, tag="pe")
                    for h in range(2):
                        nc.tensor.matmul(
                            pe[:, h],
                            outwBD[64 * h:64 * h + 64, cl],
                            rt[64 * h:64 * h + 64, ib],
                            start=True,
                            stop=True,
                        )
                    xe = stream.tile([128, 2, 512], F16, tag="xe", bufs=7)
                    x2s = x2e[:, 2 * ib:2 * ib + 2, 512 * u:512 * u + 512]
                    cv = stream.tile([128, 2, 512], F16, tag="cv", bufs=3)
                    if ib == 3 and u < 3:
                        nc.vector.tensor_copy(cv, pe)
                    else:
                        nc.scalar.copy(cv, pe)
                    xeng = nc.gpsimd if (ib == 2 and u == 1) else nc.vector
                    xeng.tensor_mul(xe, cv, x2s)
                    xes.append(xe)
                for cch in range(4):
                    for ib in range(4):
                        for j in range(2):
                            nc.tensor.matmul(
                                dbt[:, cch],
                                xes[ib][:, j, 128 * cch:128 * cch + 128],
                                s2sel,
                                start=(ib == 0 and j == 0),
                                stop=(ib == 3 and j == 1),
                            )
                nc.vector.tensor_add(
                    BstT[:, cl, 4 * u:4 * u + 4],
                    BstT[:, cl, 4 * u:4 * u + 4], dbt
                )
                # softmax numerator for this u-chunk (elementwise)
                nc.scalar.activation(
                    wT[:, cl, 4 * u:4 * u + 4],
                    BstT[:, cl, 4 * u:4 * u + 4], ACTF.Exp,
                    bias=(0.0 if shift == 0.0 else nshift3), scale=1.0,
                )
                # s~ partial sums for this u-chunk
                for gp in (2 * u, 2 * u + 1):
                    xw = stream.tile([128, 2, ID, B], BF16, tag="xw", bufs=3)
                    weng = nc.vector if (final and cl == CL - 1
                                         and gp % 2 == 1) else nc.gpsimd
                    weng.tensor_mul(
                        xw,
                        xT[:, 2 * gp:2 * gp + 2],
                        wT[:, cl, 2 * gp:2 * gp + 2]
                        .unsqueeze(2).broadcast_to((128, 2, ID, B)),
                    )
                    for gg in range(2):
                        g = 2 * gp + gg
                        for i in range(ID):
                            nc.tensor.matmul(
                                pstp(cl),
                                xw[:, gg, i],
                                rw4[:, g, i, 32 * cl:32 * cl + 32],
                                start=(gp == 0 and gg == 0 and i == 0
                                       and u == 0),
                                stop=(gp == 2 * u + 1 and gg == 1
                                      and i == ID - 1 and u == NU - 1),
                            )

            for it in range(2, n_rounds + 1):
                final = it == n_rounds
                shift = SHIFT[it]
                zps = pq[0:B, 256 + 4 * (it % 2):260 + 4 * (it % 2)]
                sqa = small.tile([B, CL], F32, tag="sqa", bufs=2)
                for pair in range(2):
                    cls = (2 * pair, 2 * pair + 1)
                    snn = small.tile([B, 2, OD], F32, tag="snn", bufs=2)
                    for cl in cls:
                        for u in range(NU):
                            dbt = pdbp.tile([128, 4, B], F32, tag="dbt")
                            eprime_unit(it, cl, u, final, shift, dbt)
                        # Z over the full row for this capsule
                        for g in range(G):
                            nc.tensor.matmul(
                                zps[:, cl:cl + 1],
                                wT[:, cl, g],
                                ones1,
                                start=(g == 0),
                                stop=(g == G - 1),
                            )
                        # normalize by Z per partition BEFORE squash so the
                        # ACT ln/exp tables see moderate ranges (sq <= ~1e3,
                        # den = 1+sq) -- the compiled tables lose precision
                        # on the unnormalized 1e28-scale values
                        zsb = small.tile([B, 1], F32, tag="zsb", bufs=4)
                        nc.vector.reciprocal(zsb, zps[:, cl:cl + 1])
                        nc.vector.tensor_mul(
                            snn[:, cl % 2], pstp(cl), zsb.broadcast_to((B, OD))
                        )
                        sqs = small.tile([B, OD], F32, tag="sqs", bufs=4)
                        nc.scalar.activation(sqs, snn[:, cl % 2], ACTF.Square,
                                             accum_out=sqa[:, cl:cl + 1])
                    # squash for the pair; alpha = sqrt(sq)/(1+sq)
                    lo, hi = cls
                    lnq = small.tile([B, 2], F32, tag="lnq", bufs=4)
                    nc.scalar.activation(lnq, sqa[:, lo:hi + 1], ACTF.Ln)
                    rs = small.tile([B, 2], F32, tag="rs", bufs=4)
                    nc.scalar.activation(rs, lnq, ACTF.Exp, bias=0.0,
                                         scale=0.5)
                    den = small.tile([B, 2], F32, tag="den", bufs=4)
                    nc.vector.tensor_scalar_add(den, sqa[:, lo:hi + 1], 1.0)
                    nc.vector.reciprocal(den, den)
                    al2 = small.tile([B, 2], F32, tag="al2", bufs=4)
                    nc.vector.tensor_mul(al2, rs, den)
                    if final:
                        for c2 in cls:
                            nc.vector.tensor_mul(
                                outf[:, c2], snn[:, c2 % 2],
                                al2[:, c2 - lo:c2 - lo + 1]
                                .broadcast_to((B, OD)),
                            )
                        nc.sync.dma_start(
                            out=out_d[:, 64 * pair:64 * pair + 64],
                            in_=outf[:, lo:hi + 1]
                            .rearrange("b c o -> b (c o)"),
                        )
                    else:
                        for c2 in cls:
                            ob = small.tile([B, OD], F32, tag="ob", bufs=4)
                            nc.vector.tensor_mul(
                                ob, snn[:, c2 % 2],
                                al2[:, c2 - lo:c2 - lo + 1]
                                .broadcast_to((B, OD))
                            )
                            pto = pq[0:OD, 264 + 64 * (c2 % 2):
                                     264 + 64 * (c2 % 2) + B]
                            nc.tensor.transpose(pto, ob, id64)
                            otn = small.tile([OD, B], F16, tag="otn",
                                             bufs=4)
                            nc.scalar.copy(otn, pto)
                            build_outw(otn, c2)
                if debug and final:
                    nc.sync.dma_start(out=dbg_B_d[:, :, :, :], in_=BstT)
                    nc.sync.dma_start(out=dbg_w_d[:, :, :, :], in_=wT)
                    zc = small.tile([B, CL], F32, tag="zc")
                    nc.scalar.copy(zc, zps)
                    nc.sync.dma_start(out=dbg_Z_d[:, :], in_=zc)

    return nc


_NC_CACHE = {}


def _get_nc(n_rounds=3, debug=False):
    key = (n_rounds, debug)
    if key not in _NC_CACHE:
        nc = _build(n_rounds=n_rounds, debug=debug)
        nc.finalize()
        _NC_CACHE[key] = nc
    return _NC_CACHE[key]


def make_in_maps(x, rw):
    x = np.asarray(x, dtype=np.float32)
    rw = np.asarray(rw, dtype=np.float32)
    # xT [128, G, ID, B]: (p, g, i, b) = x[b, 128g+p, i]
    xT_h = np.ascontiguousarray(
        x.reshape(B, G, 128, ID).transpose(2, 1, 3, 0).astype(f16)
    )
    # x2e [128, 8, N]: q<64 -> x[q, n, 2k]; q>=64 -> x[q-64, n, 2k+1]
    x2e_h = np.empty((128, 8, N), dtype=f16)
    xt = x.transpose(2, 0, 1).astype(f16)  # [i, b, n]
    for k in range(8):
        x2e_h[:64, k] = xt[2 * k]
        x2e_h[64:, k] = xt[2 * k + 1]

    in_maps = []
    for core in range(NCORES):
        rws = rw[CL * core: CL * core + CL]  # [4, N, ID, OD]
        rw4_h = np.ascontiguousarray(
            rws.reshape(CL, G, 128, ID, OD).transpose(2, 1, 3, 0, 4)
            .reshape(128, G, ID, CL * OD).astype(f16)
        )
        # rwt [cl, 32r+o, ib, n] = rw[cl, n, 4ib+r, o]
        rwt_h = np.ascontiguousarray(
            rws.reshape(CL, N, 4, 4, OD).transpose(0, 3, 4, 2, 1)
            .reshape(CL, 128, 4, N).astype(f16)
        )
        in_maps.append({"xT": xT_h, "rw4": rw4_h, "rwt": rwt_h, "x2e": x2e_h})
    return in_maps


def kernel(x, route_weights, ncores=NCORES, trace=False, n_rounds=3,
           debug=False):
    in_maps = make_in_maps(x, route_weights)
    nc = _get_nc(n_rounds=n_rounds, debug=debug)
    res = run_bass_kernel_spmd(nc, in_maps[:ncores], core_ids=list(range(ncores)),
                               trace=trace)
    if trace:
        print(f"HW exec time: {res.exec_time_ns} ns")
    if debug:
        return res.results
    outs = [r["out"].reshape(B, CL, OD) for r in res.results]
    return np.concatenate(outs, axis=1).astype(np.float32)


if __name__ == "__main__":
    rng = np.random.default_rng(0)
    x = rng.standard_normal((B, N, ID), dtype=np.float32)
    rw = rng.standard_normal((C, N, ID, OD), dtype=np.float32)
    out = kernel(x, rw)
    print(out.shape, out.dtype, float(np.abs(out).mean()))
